# revision 33
# baseline (speedup 1.0000x reference)
"""Trainium2 Bass kernel for nn_Attend_58815282151496.

Attention with l2-distance score modification + key-padding mask:
    sim = 2*scale*(q@k^T) - ||q||^2 - ||k||^2   (scale = D^-0.5)
    sim[masked j] = -inf;  out = softmax_j(sim) @ v

Key algebraic facts exploited:
  * softmax over j is invariant to per-row (per-i) constants, so the
    -||q_i||^2 term drops out entirely.
  * a global shift C keeps exp() in fp32 range without a max pass.
  * exp(0.25*qk + C - k_j^2 + mask_j) factors as
        exp(0.25*qk) * e_j,   e_j = exp(C - k_j^2 + mask_j)
    and e_j folds into the PV weights:  V'[j,:] = e_j * V[j,:].
    The ACT exp is then bias-free, so ONE ACTIVATE instruction covers
    both heads of a pair.
  * keys with mask_j > 0 contribute zero columns -> compacted away on
    the host (varlen unpad). ~half the keys drop.
  * the ACT engine (exp @ ~1.15us per [128,1024] tile) is the
    bottleneck; every 3rd j-tile's exp is offloaded to the idle DVE as
    a single tensor_scalar: i16(A*x + B) bit-cast as bf16 is a
    Schraudolph piecewise-linear 2^u approximation written directly in
    bf16 bit-pattern form (one DVE pass, ~1.2us, runs concurrently
    with ACT).

Layout strategy (all-transposed, "S^T" form), per (head-pair, i-blk, j):
  * S^T[j, i]  = kT.T @ qT  for head A into psum cols [0,512), head B
                 into cols [512,1024) (PE, bf16; heads ride disjoint
                 64-row groups -> the two matmuls run concurrently)
  * P^T[j, i]  = Exp(0.25 * S^T)      (ACT, or DVE-Schraudolph)
  * O^T[d, i] += V'_aug.T @ P^T       (PE, bf16; V' carries e_j and a
                 trailing e_j column so psum row D is the denominator)
  * out        = O^T[0:D] * (1/denom) (recip straight off PSUM + DRAM
                 bounce partition broadcast + DVE multiply)

Sharding: 32 (b,h) heads -> 4 heads per core (one pair per batch).
Host does layout-only prep: transposes, dtype casts, and gathering the
unmasked key columns.
"""

import os

import numpy as np
import ml_dtypes

import concourse.bass as bass
import concourse.bacc as bacc
import concourse.mybir as mybir
import concourse.tile as tile
from concourse.bass_utils import run_bass_kernel_spmd

B, H, N, D = 2, 16, 2048, 64
NCORES = 8
HPC = (B * H) // NCORES          # heads per core = 4
P = 128                          # partitions per j-tile
IBLK = 512                       # i-block (one psum bank per head)
NIB = N // IBLK                  # 4 i-blocks
SCALE = 2.0 * (D ** -0.5)        # 0.25, folded into ACT scale
SHIFT = 64.0                     # softmax-invariant stabilizer
NEG = -1.0e38                    # additive mask value

# Schraudolph bit-trick exp on the DVE: i16(A*x + B) bitcast to bf16 is
# a piecewise-linear 2^u approx of exp(SCALE*x); c centers the ripple
# (max ~3.3% rel, near-zero-mean after softmax normalization).
LOG2E = 1.4426950408889634
SCHR_C = -0.044
SCHR_A = 128.0 * SCALE * LOG2E
SCHR_B = (127.0 + SCHR_C) * 128.0

EXP_DVE_MOD = int(os.environ.get("K_DVE", "0"))                  # j % MOD == MOD-1 tiles exp on DVE (0=off)
RECIP_PSUM = os.environ.get("K_RP", "1") == "1"                # reciprocal_approx_fast directly from PSUM

F32 = mybir.dt.float32
BF16 = mybir.dt.bfloat16
FP16 = mybir.dt.float16
I16 = mybir.dt.int16
I32 = mybir.dt.int32
BF16NP = ml_dtypes.bfloat16

# Results of the last run (exec_time_ns etc.) for the local test harness.
LAST_RESULTS = {}


def build_bass(njts):
    """Build the per-core program; njts[pr] = j-tiles for head-pair pr."""
    njtm = max(njts)
    nkpm = njtm * P
    npairs = len(njts)
    hpc = 2 * npairs
    nc = bacc.Bacc("TRN2", target_bir_lowering=False, debug=False)

    # Per-pair packed inputs: the two heads of a pair are contiguous so
    # each load is ONE 128-partition DMA.
    qT = nc.dram_tensor("qT", [npairs, 2 * D, N], BF16, kind="ExternalInput").ap()
    kT = nc.dram_tensor("kT", [npairs, 2 * D, nkpm], BF16, kind="ExternalInput").ap()
    kc = nc.dram_tensor("kc", [npairs, P, 2, njtm, D], FP16, kind="ExternalInput").ap()
    vc = nc.dram_tensor("vc", [npairs, P, 2, njtm, D], BF16, kind="ExternalInput").ap()
    maskt = nc.dram_tensor("maskt", [npairs, P, njtm], I32, kind="ExternalInput").ap()
    oT = nc.dram_tensor("oT", [hpc, D, N], BF16, kind="ExternalOutput").ap()

    with tile.TileContext(nc) as tc:
        with (
            tc.tile_pool(name="const", bufs=1) as const_pool,
            tc.tile_pool(name="head", bufs=2) as head_pool,
            tc.tile_pool(name="pT", bufs=16) as p_pool,
            tc.tile_pool(name="spsum", bufs=2, space="PSUM") as s_psum,
            tc.tile_pool(name="opsum", bufs=2, space="PSUM") as o_psum,
            tc.tile_pool(name="outp", bufs=3) as out_pool,
            tc.tile_pool(name="epi", bufs=4) as ep_pool,
            tc.tile_pool(name="dram", bufs=2, space="DRAM") as dram_pool,
        ):
            # Preload the ACT exp table-set while the input DMAs run.
            warm = const_pool.tile([1, 1], F32)
            nc.vector.memset(warm, 0.0)
            warm2 = const_pool.tile([1, 1], F32)
            nc.scalar.activation(
                out=warm2, in_=warm, func=mybir.ActivationFunctionType.Exp)
            ones16 = const_pool.tile([1, D], BF16)
            nc.vector.memset(ones16, 1.0)


            # Deferred epilogues, flushed in two stages a few j-tiles into
            # the NEXT i-block so neither the denominator/recip chain nor
            # the DRAM-bounce latency ever head-blocks the DVE queue.
            pending1 = []   # awaiting stage 1 (recip + bounce start)
            pending2 = []   # awaiting stage 2 (multiply + store)

            def flush_stage1():
                while pending1:
                    o_ps, h, ib = pending1.pop(0)
                    denom = ep_pool.tile([1, IBLK], F32, tag="denom",
                                         name=f"dn{h}_{ib}")
                    nc.vector.tensor_copy(out=denom, in_=o_ps[D:D + 1, :])
                    recip = ep_pool.tile([1, IBLK], F32, tag="recip",
                                         name=f"rc{h}_{ib}")
                    nc.vector.reciprocal_approx_fast(out=recip, in_=denom)
                    # SBUF APs can't have zero-stride partitions; bounce the
                    # recip row through DRAM, whose APs can broadcast-read
                    recip_dram = dram_pool.tile([1, IBLK], F32, tag="rd",
                                                name=f"rd{h}_{ib}")
                    nc.gpsimd.dma_start(out=recip_dram, in_=recip)
                    recip_bc = ep_pool.tile([D, IBLK], F32, tag="recipbc",
                                            name=f"rb{h}_{ib}")
                    nc.gpsimd.dma_start(
                        out=recip_bc,
                        in_=bass.AP(
                            tensor=recip_dram.tensor, offset=recip_dram.offset,
                            ap=[[0, D], [1, IBLK]],
                        ),
                    )
                    pending2.append((o_ps, recip_bc, h, ib))

            def flush_stage2():
                while pending2:
                    o_ps, recip_bc, h, ib = pending2.pop(0)
                    ot = out_pool.tile([D, IBLK], BF16, tag="ot",
                                       name=f"ot{h}_{ib}")
                    nc.vector.tensor_tensor(
                        out=ot, in0=o_ps[0:D, :], in1=recip_bc,
                        op=mybir.AluOpType.mult,
                    )
                    nc.sync.dma_start(
                        out=oT[h, :, ib * IBLK:(ib + 1) * IBLK], in_=ot)

            def flush_pending():
                flush_stage1()
                flush_stage2()

            def epilogue(o_ps, h, ib):
                """Normalize O^T by the denominator row (psum row D)."""
                pending1.append((o_ps, h, ib))

            def epilogue_last2(o_both, hs, ib):
                """Final-i-block epilogue for BOTH heads, fully interleaved so
                the two serial chains overlap on the DVE.  The DRAM round-trip
                broadcast latency would sit on the kernel tail -- use an
                idle-PE rank-1 matmul (ones^T @ recip) into a free S-ring bank
                instead."""
                dns, rcs, rbs = [], [], []
                for o_ps, h in zip(o_both, hs):
                    # denominator copies ride the now-idle ACT engine so the
                    # DVE chain starts with the reciprocals immediately
                    dn = ep_pool.tile([1, IBLK], F32, tag="denom",
                                      name=f"dn{h}_{ib}")
                    nc.scalar.copy(out=dn, in_=o_ps[D:D + 1, :])
                    dns.append(dn)
                for dn, h in zip(dns, hs):
                    rc = ep_pool.tile([1, IBLK], F32, tag="recip",
                                      name=f"rc{h}_{ib}")
                    nc.vector.reciprocal_approx_fast(out=rc, in_=dn)
                    rcs.append(rc)
                bc_ps = s_psum.tile([P, 2 * IBLK], F32, tag="s",
                                    name=f"bc{ib}")
                for hx, (rc, h) in enumerate(zip(rcs, hs)):
                    # bf16 rank-1 broadcast: fp32 matmul would run LOW/HIGH
                    # split at ~4x the cost right on the kernel tail
                    rcb = ep_pool.tile([1, IBLK], BF16, tag="recipb",
                                       name=f"rcb{h}_{ib}")
                    nc.vector.tensor_copy(out=rcb, in_=rc)
                    nc.tensor.matmul(
                        bc_ps[0:D, hx * IBLK:(hx + 1) * IBLK],
                        lhsT=ones16, rhs=rcb, start=True, stop=True,
                    )
                    rb = ep_pool.tile([D, IBLK], F32, tag="recipbc",
                                      name=f"rb{h}_{ib}")
                    # PSUM->SBUF staging on ACT, concurrent with DVE mults
                    nc.scalar.copy(
                        out=rb, in_=bc_ps[0:D, hx * IBLK:(hx + 1) * IBLK])
                    rbs.append(rb)
                for o_ps, rb, h in zip(o_both, rbs, hs):
                    ot = out_pool.tile([D, IBLK], BF16, tag="ot",
                                       name=f"ot{h}_{ib}")
                    nc.vector.tensor_tensor(
                        out=ot, in0=o_ps[0:D, :], in1=rb,
                        op=mybir.AluOpType.mult,
                    )
                    nc.sync.dma_start(
                        out=oT[h, :, ib * IBLK:(ib + 1) * IBLK], in_=ot)

            # Trailing PVs of each block, emitted just after the next
            # block's first QK (see below).
            deferred_pvs = []

            # Heads are processed in pairs: head A lives on partitions 0-63,
            # head B on 64-127 (disjoint PE row groups -> concurrent QK).
            for pr in range(npairs):
                njt = njts[pr]
                nkp = njt * P
                ha, hb = 2 * pr, 2 * pr + 1
                # Critical-path DMAs (gate the first QK) on the sync HWDGE
                # queue; everything else on the GpSimd SWDGE queue so the
                # sync queue's ~0.6us-per-trigger cost never delays them.
                # Critical loads in arrival-deadline order: qT0 gates the
                # very first QK; kT arrives in 3 chunks sized so chunk c
                # lands just before the exp stream reaches its j-tiles.
                qT2 = head_pool.tile([2 * D, N], BF16, tag="qT")
                nc.sync.dma_start(out=qT2[:, 0:IBLK], in_=qT[pr, :, 0:IBLK])
                kT2 = head_pool.tile([2 * D, nkpm], BF16, tag="kT")
                nc.sync.dma_start(out=kT2[:, 0:2 * P], in_=kT[pr, :, 0:2 * P])
                nc.sync.dma_start(out=kT2[:, 2 * P:5 * P], in_=kT[pr, :, 2 * P:5 * P])
                nc.sync.dma_start(out=kT2[:, 5 * P:nkp], in_=kT[pr, :, 5 * P:nkp])
                kc2 = head_pool.tile([P, 2, njtm, D], FP16, tag="kc",
                                     name=f"kc{pr}")
                nc.gpsimd.dma_start(out=kc2[:, :, 0:njt, :],
                                    in_=kc[pr, :, :, 0:njt, :])
                mask_i = head_pool.tile([P, njtm], I32, tag="mi", name=f"mi{pr}")
                nc.gpsimd.dma_start(out=mask_i[:, 0:njt], in_=maskt[pr, :, 0:njt])
                vc2 = head_pool.tile([P, 2, njtm, D], BF16, tag="vs",
                                     name=f"vs{pr}")
                nc.gpsimd.dma_start(out=vc2[:, :, 0:njt, :],
                                    in_=vc[pr, :, :, 0:njt, :])
                nc.gpsimd.dma_start(out=qT2[:, IBLK:N], in_=qT[pr, :, IBLK:N])

                # maskS = SHIFT on live keys, ~NEG on masked/pad slots
                maskS = head_pool.tile([P, njtm], F32, tag="ma", name=f"ma{pr}")
                nc.vector.tensor_scalar(
                    out=maskS[:, 0:njt], in0=mask_i[:, 0:njt],
                    scalar1=NEG, scalar2=SHIFT,
                    op0=mybir.AluOpType.mult, op1=mybir.AluOpType.add,
                )

                # k^2 -> bias for BOTH heads in batched ops, one exp -> e_j
                kc_sq = head_pool.tile([P, 2, njtm, D], F32, tag="ks",
                                       name=f"ks{pr}")
                nc.vector.tensor_mul(
                    kc_sq[:, :, 0:njt, :], kc2[:, :, 0:njt, :],
                    kc2[:, :, 0:njt, :])
                k2 = head_pool.tile([P, 2, njtm], F32, tag="k2", name=f"k2{pr}")
                nc.vector.reduce_sum(
                    out=k2[:, :, 0:njt], in_=kc_sq[:, :, 0:njt, :],
                    axis=mybir.AxisListType.X)
                biasAB = head_pool.tile(
                    [P, 2, njtm], F32, tag="bi", name=f"bi{pr}")
                for hx in range(2):
                    nc.vector.tensor_sub(
                        biasAB[:, hx, 0:njt], maskS[:, 0:njt], k2[:, hx, 0:njt])
                ebAB = head_pool.tile(
                    [P, 2, njtm], F32, tag="eb", name=f"eb{pr}")
                v_aug = head_pool.tile([P, 2, njtm, D + 1], BF16, tag="va",
                                       name=f"va{pr}")

                def emit_vprep():
                    # Emitted mid-loop (after a few exps are queued) so this
                    # chain never head-of-line-blocks the ACT/DVE queues.
                    nc.scalar.activation(
                        out=ebAB[:, :, 0:njt], in_=biasAB[:, :, 0:njt],
                        func=mybir.ActivationFunctionType.Exp,
                    )
                    # ONE batched multiply per head: e_j broadcasts along D
                    # via a zero-stride free dim on the in1 read AP.
                    for hx in range(2):
                        nc.vector.tensor_copy(
                            out=v_aug[:, hx, 0:njt, D], in_=ebAB[:, hx, 0:njt])
                        e_sl = ebAB[:, hx, 0:njt]
                        e_bc = bass.AP(
                            tensor=e_sl.tensor, offset=e_sl.offset,
                            ap=[list(p) for p in e_sl.ap] + [[0, D]],
                        )
                        nc.vector.tensor_tensor(
                            out=v_aug[:, hx, 0:njt, 0:D],
                            in0=vc2[:, hx, 0:njt, :],
                            in1=e_bc,
                            op=mybir.AluOpType.mult,
                        )

                for ib in range(NIB):
                    # PV lag on the first i-block gives the V' chain deadline
                    # slack; later blocks keep the tight PV-after-exp order
                    # (a lag on the last block would lengthen the tail).
                    LAG = min(5, njt - 1) if ib == 0 else 0
                    last_blk_ = (pr == npairs - 1 and ib == NIB - 1)
                    defer_n = 0 if last_blk_ else 2
                    oa = o_psum.tile([D + 1, IBLK], F32, tag="oa",
                                     name=f"oa{pr}_{ib}")
                    ob = o_psum.tile([D + 1, IBLK], F32, tag="ob",
                                     name=f"ob{pr}_{ib}")
                    o_both = (oa, ob)
                    i0 = ib * IBLK
                    pts = {}

                    def emit_pv(jj, pts=pts, o_both=o_both, v_aug=v_aug,
                                njt=njt):
                        # default-arg binding: deferred calls from the next
                        # block must see THIS block's state, not the rebound
                        # loop variables
                        for hx in range(2):
                            nc.tensor.matmul(
                                o_both[hx],
                                lhsT=v_aug[:, hx, jj, :],
                                rhs=pts[jj][:, hx * IBLK:(hx + 1) * IBLK],
                                start=(jj == 0), stop=(jj == njt - 1),
                            )
                        del pts[jj]

                    last_blk = (pr == npairs - 1 and ib == NIB - 1)
                    for j in range(njt):
                        if ib == 0 and j == 1:
                            emit_vprep()
                        if ib > 0 or pr > 0:
                            if j == 1:
                                flush_stage1()
                            elif j == 5:
                                flush_stage2()
                        s = s_psum.tile([P, 2 * IBLK], F32, tag="s",
                                        name=f"s{pr}_{ib}_{j}")
                        for hx in range(2):
                            r0 = hx * D
                            nc.tensor.matmul(
                                s[:, hx * IBLK:(hx + 1) * IBLK],
                                lhsT=kT2[r0:r0 + D, j * P:(j + 1) * P],
                                rhs=qT2[r0:r0 + D, i0:i0 + IBLK],
                                start=True, stop=True,
                            )
                        if j == 0 and deferred_pvs:
                            # previous block's trailing PVs, deferred past
                            # this block's first QK so the PE queue head is
                            # never a PV waiting on the previous exp
                            for fn in deferred_pvs:
                                fn()
                            deferred_pvs.clear()
                        pT = p_pool.tile([P, 2 * IBLK], BF16, tag="p",
                                         name=f"p{pr}_{ib}_{j}")
                        if pr == 0 and ib == 0:
                            # k2/v_prep own the DVE early; offload later tiles
                            dve_here = j % 3 == 1 and j > 3
                        else:
                            dve_here = (EXP_DVE_MOD
                                        and j % EXP_DVE_MOD == EXP_DVE_MOD - 1)
                        dve_exp = (
                            EXP_DVE_MOD
                            and dve_here
                            and not (last_blk and j >= njt - 2)
                        )
                        if dve_exp:
                            nc.vector.tensor_scalar(
                                out=pT.bitcast(I16), in0=s,
                                scalar1=SCHR_A, scalar2=SCHR_B,
                                op0=mybir.AluOpType.mult,
                                op1=mybir.AluOpType.add,
                            )
                        else:
                            nc.scalar.activation(
                                out=pT, in_=s,
                                func=mybir.ActivationFunctionType.Exp,
                                scale=SCALE,
                            )
                        pts[j] = pT
                        if j >= LAG and (j - LAG) < njt - defer_n:
                            emit_pv(j - LAG)
                    for j in range(max(0, njt - LAG), njt - defer_n):
                        emit_pv(j)
                    # the final PVs wait on this block's final exps; deferring
                    # them into the next block keeps the in-order PE queue
                    # head from stalling the next block's QKs behind them
                    for j in range(njt - defer_n, njt):
                        deferred_pvs.append((lambda jj=j, f=emit_pv: f(jj)))

                    if last_blk:
                        flush_pending()
                        epilogue_last2(o_both, (ha, hb), ib)
                    else:
                        epilogue(oa, ha, ib)
                        epilogue(ob, hb, ib)
                flush_pending() if pr == npairs - 1 else None
    nc.compile()
    return nc


_NC_CACHE = {}


def _get_nc(njts):
    if njts not in _NC_CACHE:
        _NC_CACHE[njts] = build_bass(njts)
    return _NC_CACHE[njts]


def make_in_maps(q, k, v, mask):
    """Host-side layout prep: per core one head-pair from each batch,
    unmasked-key compaction (gather), transposes, dtype casts."""
    q = np.ascontiguousarray(np.asarray(q, dtype=np.float32))
    k = np.ascontiguousarray(np.asarray(k, dtype=np.float32))
    v = np.ascontiguousarray(np.asarray(v, dtype=np.float32))
    mask = np.asarray(mask, dtype=np.int32)

    idxs = [np.flatnonzero(mask[b] <= 0) for b in range(B)]
    njts = tuple(max(1, (len(ix) + P - 1) // P) for ix in idxs)
    njtm = max(njts)
    nkpm = njtm * P

    # per-batch compacted+padded keys/values and pad mask
    kcb = np.zeros((B, H, nkpm, D), np.float32)
    vcb = np.zeros((B, H, nkpm, D), np.float32)
    mtb = np.ones((B, nkpm), np.int32)  # 1 = padding slot
    for b, ix in enumerate(idxs):
        n = len(ix)
        kcb[b, :, :n] = k[b][:, ix]
        vcb[b, :, :n] = v[b][:, ix]
        mtb[b, :n] = 0

    qTt = q.transpose(0, 1, 3, 2)                     # [B, H, D, N]
    kTt = kcb.transpose(0, 1, 3, 2)                   # [B, H, D, nkpm]
    # [P, njt, D] pre-arranged (j = t*P + p) for contiguous DMA rows
    kcr = kcb.reshape(B, H, njtm, P, D).transpose(0, 1, 3, 2, 4)
    vcr = vcb.reshape(B, H, njtm, P, D).transpose(0, 1, 3, 2, 4)
    mtr = mtb.reshape(B, njtm, P).transpose(0, 2, 1)  # [B, P, njtm]

    in_maps = []
    head_lists = []
    for c in range(NCORES):
        # one pair from each batch: batch b contributes heads (2c, 2c+1)
        heads = [(0, 2 * c), (0, 2 * c + 1), (1, 2 * c), (1, 2 * c + 1)]
        head_lists.append(heads)
        # packed per-pair arrays: pair pr = batch pr, heads (2c, 2c+1)
        qTp = np.stack([
            np.concatenate([qTt[b, 2 * c], qTt[b, 2 * c + 1]], axis=0)
            for b in range(B)])                        # [NP, 128, N]
        kTp = np.stack([
            np.concatenate([kTt[b, 2 * c], kTt[b, 2 * c + 1]], axis=0)
            for b in range(B)])                        # [NP, 128, nkpm]
        kcp = np.stack([
            np.stack([kcr[b, 2 * c], kcr[b, 2 * c + 1]], axis=1)
            for b in range(B)])                        # [NP, P, 2, njtm, D]
        vcp = np.stack([
            np.stack([vcr[b, 2 * c], vcr[b, 2 * c + 1]], axis=1)
            for b in range(B)])
        in_maps.append({
            "qT": np.ascontiguousarray(qTp).astype(BF16NP),
            "kT": np.ascontiguousarray(kTp).astype(BF16NP),
            "kc": np.ascontiguousarray(kcp).astype(np.float16),
            "vc": np.ascontiguousarray(vcp).astype(BF16NP),
            "maskt": np.ascontiguousarray(mtr[[0, 1]]),
        })
    return njts, head_lists, in_maps


def kernel(q, k, v, mask):
    njts, head_lists, in_maps = make_in_maps(q, k, v, mask)
    nc = _get_nc(njts)

    kwargs = {}
    if os.environ.get("ATT_TRACE") in ("1", "true"):
        kwargs.update(trace=True, trace_cores=[0])
        if os.environ.get("ATT_TRACE_DIR"):
            kwargs.update(tmpdir=os.environ["ATT_TRACE_DIR"])

    res = run_bass_kernel_spmd(nc, in_maps, core_ids=list(range(NCORES)), **kwargs)
    LAST_RESULTS["exec_time_ns"] = res.exec_time_ns
    LAST_RESULTS["trace"] = res.instructions_and_trace

    out = np.empty((B, H, N, D), dtype=np.float32)
    for c in range(NCORES):
        oTc = res.results[c]["oT"]  # [HPC, D, N] bf16
        for hh, (b, h) in enumerate(head_lists[c]):
            out[b, h] = oTc[hh].T.astype(np.float32)
    return out
